# revision 12
# baseline (speedup 1.0000x reference)
"""2-layer GCN on 8 Trainium2 NeuronCores — v2 (pair-gather + one-hot matmul).

Design (derived from dma_gather microbenchmarks: SWDGE descriptor dispatch
is the bottleneck at ~8.1ns/desc/queue; 512B descriptors move 2x the bytes
of 256B ones at the same descriptor rate):

  - GCN norm factorizes: z = dinv (.) ((A+I) (dinv (.) h W)), so both layers
    are transform-first: u = dinv (.) (h @ W) [N, 64] f32, then an unweighted
    row-gather + segment-sum over edges.
  - u rows are packed in PAIRS: table row m = [u[2m] | u[2m+1]], 128 f32 =
    512B — one descriptor per edge fetches the pair containing its source.
    25088 pair rows < 32768 fits int16 indices with no windowing.
  - Nodes are sharded in natural order (core c owns rows [6272c, 6272c+6272)).
    Slots (edges) are grouped per dst tile, chunked by 128; aggregation is
    S_both^T @ g via the Tensor engine: S_both [128 slots, 256] bf16 one-hot
    built on DVE with a single is_equal against an iota constant
    (code = dst_pos + 128*parity), two matmuls per chunk (even|odd half of
    the gathered pair) accumulating in PSUM f32.
  - Self-loops are free: PSUM is initialised with identity @ u_own[tile].
  - Pad slots: idx = -1 (trailing; ucode trims them — zero DMA cost) and
    code = 384 (matches nothing — zero S column kills garbage lanes).
  - Cross-core slot-count imbalance is absorbed the same way: static shapes
    are the max over cores; shorter cores' trailing -1 blocks are trimmed.
  - The u tables are AllGathered in two halves so the second half's
    collective overlaps the first half's gathers.

Host does integer index bookkeeping only; all FP math runs on device.
"""

import sys

sys.path.insert(0, "/opt/trn_rl_repo")

import numpy as np
import ml_dtypes

P = 128
CORES = 8
SLAB = 6272  # nodes per core (49 tiles)
TILES = SLAB // P  # 49
HALF = SLAB // 2  # 3136
V = CORES * SLAB  # 50176
PAIRS = V // 2  # 25088
IN_C = 128
HID_C = 64
OUT_C = 64
MAX_COLS = 32  # idx cols per dma_gather instruction (4096 idxs)
PAD_CODE = 384.0

_bf16 = ml_dtypes.bfloat16


def _prep(x, edge_index):
    n = x.shape[0]
    assert n == 50000 and x.shape[1] == IN_C
    src = np.asarray(edge_index[0], dtype=np.int64)
    dst = np.asarray(edge_index[1], dtype=np.int64)

    deg = (np.bincount(dst, minlength=n) + 1).astype(np.float32)
    # per-core [128, TILES] deg table in (partition, tile) layout; pads -> 1
    degpad = np.ones(V, dtype=np.float32)
    degpad[:n] = deg
    deg_own = degpad.reshape(CORES, TILES, P).transpose(0, 2, 1).copy()

    # table row of node s (AllGather-in-halves layout)
    c_s = src // SLAB
    r_s = src % SLAB
    h_s = r_s // HALF
    R = PAIRS * h_s + HALF * c_s + (r_s - HALF * h_s)
    m = R // 2  # pair row
    par = R % 2

    c_d = dst // SLAB
    j_d = dst % SLAB
    t_d = j_d // P
    p_d = j_d % P

    # group key: (core, tile, half-of-src, pair-row) — sort once
    order = np.lexsort((m, h_s, t_d, c_d))
    c_d, t_d, p_d, h_s, m, par = (
        a[order] for a in (c_d, t_d, p_d, h_s, m, par)
    )

    # per (core, tile, half) counts -> static max cols per (tile, half)
    gid = (c_d * TILES + t_d) * 2 + h_s
    cnt = np.bincount(gid, minlength=CORES * TILES * 2).reshape(CORES, TILES, 2)
    cols_needed = -(-cnt // P)  # ceil
    cols_static = cols_needed.max(axis=0)  # [TILES, 2]
    total_cols = int(cols_static.sum())

    # slot positions inside each (core,tile,half) group
    starts = np.zeros(CORES * TILES * 2, dtype=np.int64)
    starts[1:] = np.cumsum(cnt.reshape(-1))[:-1]
    pos_in_grp = np.arange(len(m)) - starts[gid]

    # global column offset of each (tile, half) group (same for all cores)
    gcol = np.zeros((TILES, 2), dtype=np.int64)
    flat = cols_static.reshape(-1)
    gcol.reshape(-1)[1:] = np.cumsum(flat)[:-1]

    F = total_cols * 8  # idx array free dim ([16, F], 16 idxs per col chunk)

    idx16 = np.zeros((CORES, 16, F), dtype=np.int16)
    code = np.full((CORES, P, total_cols), PAD_CODE, dtype=np.float32)

    col = gcol[t_d, h_s] + pos_in_grp // P
    lane = pos_in_grp % P
    # idx flat position within the [16, F] wrapped layout:
    # flat i = col*128 + lane ; row = i % 16, fcol = i // 16
    i_flat = col * P + lane
    idx16[c_d, i_flat % 16, i_flat // 16] = m.astype(np.int16)
    code[c_d, lane, col] = (p_d + P * par).astype(np.float32)

    # chunking of each (tile, half) group into instructions
    instrs = []  # (tile, col0, ncols, half)
    for t in range(TILES):
        for h in range(2):
            c0 = int(gcol[t, h])
            rem = int(cols_static[t, h])
            while rem > 0:
                take = min(MAX_COLS, rem)
                instrs.append((t, c0, take, h))
                c0 += take
                rem -= take

    # x slabs padded to [CORES, SLAB, IN_C]
    xpad = np.zeros((V, IN_C), dtype=np.float32)
    xpad[:n] = np.asarray(x, dtype=np.float32)
    xs = xpad.reshape(CORES, SLAB, IN_C)

    shapes = dict(
        total_cols=total_cols,
        F=F,
        instrs=instrs,
        cols_static=cols_static,
        gcol=gcol,
    )
    percore = dict(idx=idx16, code=code, deg_own=deg_own, xs=xs)
    return shapes, percore


def _build(shapes):
    from concourse import bass, bacc, mybir, tile
    from concourse.masks import make_identity

    f32 = mybir.dt.float32
    bf16 = mybir.dt.bfloat16
    i16 = mybir.dt.int16

    total_cols = shapes["total_cols"]
    F = shapes["F"]
    instrs = shapes["instrs"]

    nc = bacc.Bacc(None, target_bir_lowering=False, num_swdge_queues=4)
    xin = nc.declare_dram_parameter("xin", [SLAB, IN_C], f32, isOutput=False)
    w1 = nc.declare_dram_parameter("w1", [IN_C, HID_C], bf16, isOutput=False)
    w2 = nc.declare_dram_parameter("w2", [HID_C, OUT_C], bf16, isOutput=False)
    b1 = nc.declare_dram_parameter("b1", [P, HID_C], f32, isOutput=False)
    b2 = nc.declare_dram_parameter("b2", [P, OUT_C], f32, isOutput=False)
    idx = nc.declare_dram_parameter("idx", [16, F], i16, isOutput=False)
    code = nc.declare_dram_parameter("code", [P, total_cols], bf16, isOutput=False)
    iota = nc.declare_dram_parameter(
        "iota", [P, MAX_COLS * 2 * P], bf16, isOutput=False
    )
    deg_own = nc.declare_dram_parameter("deg_own", [P, TILES], f32, isOutput=False)
    zout = nc.declare_dram_parameter("zout", [SLAB, OUT_C], f32, isOutput=True)

    mult = mybir.AluOpType.mult
    add = mybir.AluOpType.add
    iseq = mybir.AluOpType.is_equal
    Relu = mybir.ActivationFunctionType.Relu
    Copy = mybir.ActivationFunctionType.Copy
    qctr = [0]

    def ap3(ap, shape3):
        """AP with explicit 3-level pattern (allows stride-0 broadcast)."""
        return bass.AP(ap.tensor, ap.offset, shape3)

    with tile.TileContext(nc) as tc:
        with (
            tc.tile_pool(name="const", bufs=1) as cp,
            tc.tile_pool(name="dram", bufs=1, space="DRAM") as dp,
        ):
            w1_t = cp.tile([IN_C, HID_C], bf16)
            nc.sync.dma_start(w1_t[:], w1[:])
            w2_t = cp.tile([HID_C, OUT_C], bf16)
            nc.sync.dma_start(w2_t[:], w2[:])
            b1_t = cp.tile([P, HID_C], f32)
            nc.sync.dma_start(b1_t[:], b1[:])
            b2_t = cp.tile([P, OUT_C], f32)
            nc.sync.dma_start(b2_t[:], b2[:])
            iota_t = cp.tile([P, MAX_COLS * 2 * P], bf16)
            nc.sync.dma_start(iota_t[:], iota[:])

            identf = cp.tile([P, P], f32)
            make_identity(nc, identf[:])
            identb = cp.tile([P, P], bf16)
            make_identity(nc, identb[:])

            deg_t = cp.tile([P, TILES], f32)
            nc.sync.dma_start(deg_t[:], deg_own[:])
            dinv_t = cp.tile([P, TILES], f32)
            nc.vector.reciprocal(dinv_t[:], deg_t[:])
            nc.scalar.activation(
                dinv_t[:], dinv_t[:], mybir.ActivationFunctionType.Sqrt
            )

            idx_t = cp.tile([P, F], i16)
            for g in range(8):
                nc.sync.dma_start(idx_t[16 * g : 16 * (g + 1), :], idx[:, :])
            code_t = cp.tile([P, total_cols], bf16)
            nc.sync.dma_start(code_t[:], code[:])

            u1_own = cp.tile([P, TILES, HID_C], bf16)
            u2_own = cp.tile([P, TILES, HID_C], bf16)

            u1shard = dp.tile([SLAB, HID_C], f32)
            u2shard = dp.tile([SLAB, HID_C], f32)
            table1 = dp.tile([V, HID_C], f32)
            table2 = dp.tile([V, HID_C], f32)

            def table_pairs(table):
                tb = table[:]
                return bass.AP(tb.tensor, tb.offset, [[IN_C, PAIRS], [1, IN_C]])

            # ---------- prepass: u1 = dinv (.) (x @ W1) ----------
            with (
                tc.tile_pool(name="px", bufs=3) as px,
                tc.tile_pool(name="pp", bufs=3, space="PSUM") as ppp,
                tc.tile_pool(name="ps", bufs=3) as pps,
            ):
                for t in range(TILES):
                    x_t = px.tile([P, IN_C], f32, tag="x")
                    nc.sync.dma_start(x_t[:], xin[P * t : P * (t + 1), :])
                    xT_ps = ppp.tile([IN_C, P], f32, tag="xT")
                    nc.tensor.transpose(xT_ps[:], x_t[:], identf[:])
                    xT = pps.tile([IN_C, P], bf16, tag="xTs")
                    nc.scalar.activation(xT[:], xT_ps[:], Copy)
                    u_ps = ppp.tile([P, HID_C], f32, tag="u")
                    nc.tensor.matmul(
                        out=u_ps[:], lhsT=xT[:], rhs=w1_t[:], start=True, stop=True
                    )
                    uf = pps.tile([P, HID_C], f32, tag="uf")
                    nc.vector.tensor_scalar(
                        out=uf[:],
                        in0=u_ps[:],
                        scalar1=dinv_t[:, t : t + 1],
                        scalar2=None,
                        op0=mult,
                    )
                    nc.vector.tensor_copy(u1_own[:, t, :], uf[:])
                    nc.sync.dma_start(u1shard[P * t : P * (t + 1), :], uf[:])

            # ---------- AllGather u1 (two halves) ----------
            for h in range(2):
                nc.gpsimd.collective_compute(
                    "AllGather",
                    mybir.AluOpType.bypass,
                    replica_groups=[list(range(CORES))],
                    ins=[u1shard[h * HALF : (h + 1) * HALF, :]],
                    outs=[table1[h * CORES * HALF : (h + 1) * CORES * HALF, :]],
                )

            # ---------- generic gather+aggregate layer ----------
            def layer(table, u_own, epilogue):
                with (
                    tc.tile_pool(name="g", bufs=4) as gp,
                    tc.tile_pool(name="g16", bufs=3) as gp16,
                    tc.tile_pool(name="sp", bufs=3) as sp,
                    tc.tile_pool(name="agg", bufs=4, space="PSUM") as aggp,
                    tc.tile_pool(name="tps", bufs=2, space="PSUM") as tpp,
                    tc.tile_pool(name="eps", bufs=2) as eps,
                ):
                    tbl = table_pairs(table)
                    cur_tile = -1
                    psum = None
                    by_tile = {}
                    for t, c0, ncols, h in instrs:
                        by_tile.setdefault(t, []).append((c0, ncols, h))
                    for t in range(TILES):
                        psum = aggp.tile([P, HID_C], f32, tag="agg")
                        nc.tensor.matmul(
                            out=psum[:],
                            lhsT=identb[:],
                            rhs=u_own[:, t, :],
                            start=True,
                            stop=False,
                        )
                        chunks = by_tile[t]
                        nch = sum(nc_ for _, nc_, _ in chunks)
                        done = 0
                        for c0, ncols, h in chunks:
                            g_t = gp.tile([P, MAX_COLS, IN_C], f32, tag="g")
                            nc.gpsimd.dma_gather(
                                out_ap=g_t[:, :ncols, :],
                                in_ap=tbl,
                                idxs_ap=idx_t[:, 8 * c0 : 8 * (c0 + ncols)],
                                num_idxs=P * ncols,
                                num_idxs_reg=P * ncols,
                                elem_size=IN_C,
                                single_packet=False,
                                queue_num=qctr[0] % 4,
                            )
                            qctr[0] += 1
                            g16_t = gp16.tile([P, MAX_COLS, IN_C], bf16, tag="g16")
                            nc.scalar.activation(
                                g16_t[:, :ncols, :], g_t[:, :ncols, :], Copy
                            )
                            s_t = sp.tile([P, MAX_COLS, 2 * P], bf16, tag="s")
                            nc.vector.tensor_tensor(
                                out=s_t[:, :ncols, :],
                                in0=ap3(
                                    iota_t[:],
                                    [
                                        list(iota_t[:].ap[0]),
                                        [2 * P, ncols],
                                        [1, 2 * P],
                                    ],
                                ),
                                in1=ap3(
                                    code_t[:, c0 : c0 + ncols],
                                    [
                                        list(code_t[:].ap[0]),
                                        [1, ncols],
                                        [0, 2 * P],
                                    ],
                                ),
                                op=iseq,
                            )
                            for j in range(ncols):
                                done += 1
                                last = done == nch
                                nc.tensor.matmul(
                                    out=psum[:],
                                    lhsT=s_t[:, j, 0:P],
                                    rhs=g16_t[:, j, 0:HID_C],
                                    start=False,
                                    stop=False,
                                )
                                nc.tensor.matmul(
                                    out=psum[:],
                                    lhsT=s_t[:, j, P : 2 * P],
                                    rhs=g16_t[:, j, HID_C:IN_C],
                                    start=False,
                                    stop=last,
                                )
                        epilogue(t, psum, eps, tpp)

            # ---------- layer 1 epilogue: h=relu(dinv*agg+b1); u2=dinv*(h@W2) ----------
            def epi1(t, psum, eps, tpp):
                h_t = eps.tile([P, HID_C], f32, tag="h")
                nc.vector.scalar_tensor_tensor(
                    out=h_t[:],
                    in0=psum[:],
                    scalar=dinv_t[:, t : t + 1],
                    in1=b1_t[:],
                    op0=mult,
                    op1=add,
                )
                hr = eps.tile([P, HID_C], f32, tag="hr")
                nc.scalar.activation(hr[:], h_t[:], Relu)
                hT_ps = tpp.tile([HID_C, P], f32, tag="hT")
                nc.tensor.transpose(hT_ps[:], hr[:], identf[:])
                hT = eps.tile([HID_C, P], bf16, tag="hTs")
                nc.scalar.activation(hT[:], hT_ps[:], Copy)
                u_ps = tpp.tile([P, HID_C], f32, tag="u2")
                nc.tensor.matmul(
                    out=u_ps[:], lhsT=hT[:], rhs=w2_t[:], start=True, stop=True
                )
                uf = eps.tile([P, HID_C], f32, tag="u2f")
                nc.vector.tensor_scalar(
                    out=uf[:],
                    in0=u_ps[:],
                    scalar1=dinv_t[:, t : t + 1],
                    scalar2=None,
                    op0=mult,
                )
                nc.vector.tensor_copy(u2_own[:, t, :], uf[:])
                nc.sync.dma_start(u2shard[P * t : P * (t + 1), :], uf[:])

            layer(table1, u1_own, epi1)

            # ---------- AllGather u2 (two halves) ----------
            for h in range(2):
                nc.gpsimd.collective_compute(
                    "AllGather",
                    mybir.AluOpType.bypass,
                    replica_groups=[list(range(CORES))],
                    ins=[u2shard[h * HALF : (h + 1) * HALF, :]],
                    outs=[table2[h * CORES * HALF : (h + 1) * CORES * HALF, :]],
                )

            # ---------- layer 2 epilogue: z = dinv*agg + b2 ----------
            def epi2(t, psum, eps, tpp):
                z_t = eps.tile([P, OUT_C], f32, tag="z")
                nc.vector.scalar_tensor_tensor(
                    out=z_t[:],
                    in0=psum[:],
                    scalar=dinv_t[:, t : t + 1],
                    in1=b2_t[:],
                    op0=mult,
                    op1=add,
                )
                nc.sync.dma_start(zout[P * t : P * (t + 1), :], z_t[:])

            layer(table2, u2_own, epi2)

    nc.finalize()
    return nc


def make_program(x, edge_index, W1, b1, W2, b2):
    x = np.ascontiguousarray(np.asarray(x, dtype=np.float32))
    shapes, percore = _prep(x, edge_index)
    nc = _build(shapes)

    iota_arr = np.broadcast_to(
        np.tile(np.arange(2 * P, dtype=np.float32), MAX_COLS),
        (P, MAX_COLS * 2 * P),
    ).astype(_bf16)
    b1_bc = np.ascontiguousarray(
        np.broadcast_to(np.asarray(b1, np.float32), (P, HID_C))
    )
    b2_bc = np.ascontiguousarray(
        np.broadcast_to(np.asarray(b2, np.float32), (P, OUT_C))
    )
    W1a = np.ascontiguousarray(np.asarray(W1, np.float32).astype(_bf16))
    W2a = np.ascontiguousarray(np.asarray(W2, np.float32).astype(_bf16))

    in_maps = []
    for c in range(CORES):
        in_maps.append(
            {
                "xin": np.ascontiguousarray(percore["xs"][c]),
                "w1": W1a,
                "w2": W2a,
                "b1": b1_bc,
                "b2": b2_bc,
                "idx": np.ascontiguousarray(percore["idx"][c]),
                "code": np.ascontiguousarray(
                    percore["code"][c].astype(_bf16)
                ),
                "iota": np.ascontiguousarray(iota_arr),
                "deg_own": np.ascontiguousarray(percore["deg_own"][c]),
            }
        )
    return nc, in_maps


def kernel(x, edge_index, W1, b1, W2, b2):
    from concourse.bass_utils import run_bass_kernel_spmd

    n = np.asarray(x).shape[0]
    nc, in_maps = make_program(x, edge_index, W1, b1, W2, b2)
    res = run_bass_kernel_spmd(nc, in_maps, list(range(CORES)))

    z = np.empty((n, OUT_C), dtype=np.float32)
    for c in range(CORES):
        lo = c * SLAB
        hi = min(lo + SLAB, n)
        z[lo:hi] = res.results[c]["zout"][: hi - lo]
    return z
